# revision 14
# baseline (speedup 1.0000x reference)
"""AnomalyAttention (two causal attentions per (b,h)) on 8 TRN2 NeuronCores.

Sharding: B*H = 16 (batch, head) pairs -> 2 pairs per core. Each core runs
4 independent causal attentions (time + channel for each of its 2 pairs).
No cross-core communication.

Layout ("transposed PV" + dual-engine exp), bf16 matmuls:
  S^T[k, q]   = kT_tile.T @ qT        (PE bf16, contraction E=64; the two
                attn types use PE row groups 0-63 / 64-127)
  P^T         = exp(scale * S^T)      two paths:
                - ACT exp for pass-0 and pass-1 early key tiles
                - DVE Schraudolph for pass-1 k>=8: tensor_scalar
                  (x*a+b -> int32 = float bits of exp(x)) + bitcast
                  tensor_copy -> bf16 (~3.5% rel err, but only on the
                  late-key mass of late queries whose outputs are small)
  diag mask   = GPSIMD affine_select zero-triangle on the diag block
  out[q, d]  += P^T[k, q-blk].T @ V_ext[k, d]   (PE bf16, 65-col moving;
                PV emitted two chunks behind exp so PE instructions queue
                with satisfied deps)
V_ext carries a ones column -> out col 64 = softmax denominator per query
partition. Epilogue per completed PSUM bank: DVE reciprocal + per-q-tile
tensor_scalar multiply into a staging tile; one contiguous DMA per 8
q-tiles. PSUM discipline: start=True clears has_written for the WHOLE
bank, so exactly one start per bank (first matmul emitted into it).
"""

import math
from contextlib import ExitStack

import ml_dtypes
import numpy as np

import concourse.bacc as bacc
import concourse.mybir as mybir
import concourse.tile as tile
from concourse.bass_utils import run_bass_kernel_spmd

B, L, H, E, D = 2, 2048, 8, 64, 64
NCORES = 8
PAIRS = (B * H) // NCORES          # (b,h) pairs per core = 2
NATT = 2 * PAIRS                   # attentions per core = 4
SCALE = 1.0 / math.sqrt(E)
P = 128                            # partitions / key-tile size
NKT = L // P                       # 16 key tiles
HALF = L // 2                      # query half-pass size (PSUM budget)
DP1 = D + 1                        # value cols + denominator ones-column
F32 = mybir.dt.float32
I32 = mybir.dt.int32
BF16 = mybir.dt.bfloat16
FP8 = mybir.dt.float8e4

# fp8 DoubleRow scores only for pass 1 (query rows >= 1024): early rows
# have few softmax terms and large outputs, so they keep bf16 exactness;
# late rows average over >=1024 keys and absorb the fp8 quantization.
USE_FP8_SCORES = False
# Schraudolph exp on DVE for pass-1 key tiles >= KD (late keys of late
# queries; their extra ~3.5% relative error lands on the rows with the
# smallest output magnitude)
KD = 10

LOG2E = 1.4426950408889634
SCH_A = SCALE * LOG2E * (1 << 23)
SCH_B = float((127 << 23) - 298634)

_CACHE = {}


def _build_nc():
    nc = bacc.Bacc()
    qt = nc.declare_dram_parameter("qt", [P, PAIRS, L], BF16, isOutput=False)
    kt = nc.declare_dram_parameter("kt", [P, PAIRS, L], BF16, isOutput=False)
    if USE_FP8_SCORES:
        # E-dim folded as [32 partitions, 2 sub-rows] for DoubleRow
        qt8 = nc.declare_dram_parameter("qt8", [64, PAIRS, 2, HALF], FP8, isOutput=False)
        kt8 = nc.declare_dram_parameter("kt8", [64, PAIRS, 2, L], FP8, isOutput=False)
    else:
        qt8 = kt8 = None
    ve = nc.declare_dram_parameter("ve", [P, NATT, NKT, DP1], BF16, isOutput=False)
    out = nc.declare_dram_parameter("out", [NATT, P, NKT, D], F32, isOutput=True)

    with tile.TileContext(nc) as tc:
        with ExitStack() as ctx:
            _body(ctx, tc, qt, kt, qt8, kt8, ve, out)
    nc.finalize()
    return nc


def _body(ctx, tc, qt, kt, qt8, kt8, ve, out):
    nc = tc.nc
    Exp = mybir.ActivationFunctionType.Exp

    persist = ctx.enter_context(tc.tile_pool(name="persist", bufs=1))
    s_psum = ctx.enter_context(tc.tile_pool(name="s_psum", bufs=2, space="PSUM"))
    pv_psum = ctx.enter_context(tc.tile_pool(name="pv_psum", bufs=1, space="PSUM"))
    p_pool = ctx.enter_context(tc.tile_pool(name="p_pool", bufs=4))
    i_pool = ctx.enter_context(tc.tile_pool(name="i_pool", bufs=3))
    e_pool = ctx.enter_context(tc.tile_pool(name="e_pool", bufs=4))

    # warm the ACT exp table before any real dependency exists
    warm = persist.tile([1, 8], F32)
    nc.vector.memset(warm, 0.0)
    nc.scalar.activation(warm, warm, Exp)


    # triangular mask tile: 1 where q >= key else 0 (built once)
    mask = persist.tile([P, P], BF16)
    nc.vector.memset(mask, 1.0)
    nc.gpsimd.affine_select(
        out=mask, in_=mask, compare_op=mybir.AluOpType.is_ge,
        fill=0.0, base=0, channel_multiplier=-1, pattern=[[1, P]],
    )

    qt_sb = persist.tile([P, PAIRS, L], BF16)
    kt_sb = persist.tile([P, PAIRS, L], BF16)
    if USE_FP8_SCORES:
        qt8_sb = persist.tile([64, PAIRS, 2, HALF], FP8)
        kt8_sb = persist.tile([64, PAIRS, 2, L], FP8)
    ve_sb = persist.tile([P, NATT, NKT, DP1], BF16)

    # staged input DMA: the three pieces the first chunk needs go out on
    # three different engines' queues in parallel
    nc.gpsimd.dma_start(out=kt_sb[:, 0, 0:P], in_=kt[:, 0, 0:P])
    nc.scalar.dma_start(out=qt_sb[:, 0, 0:512], in_=qt[:, 0, 0:512])
    nc.sync.dma_start(out=qt_sb[:, 0, 512:HALF], in_=qt[:, 0, 512:HALF])
    nc.gpsimd.dma_start(out=ve_sb[:, 0], in_=ve[:, 0])
    nc.gpsimd.dma_start(out=ve_sb[:, 1], in_=ve[:, 1])
    nc.sync.dma_start(out=kt_sb[:, 0, P:L], in_=kt[:, 0, P:L])
    nc.scalar.dma_start(out=qt_sb[:, 0, HALF:L], in_=qt[:, 0, HALF:L])
    if USE_FP8_SCORES:
        nc.sync.dma_start(out=kt8_sb[:, 0], in_=kt8[:, 0])
        nc.scalar.dma_start(out=qt8_sb[:, 0], in_=qt8[:, 0])
    nc.sync.dma_start(out=kt_sb[:, 1], in_=kt[:, 1])
    nc.scalar.dma_start(out=qt_sb[:, 1], in_=qt[:, 1])
    if USE_FP8_SCORES:
        nc.sync.dma_start(out=kt8_sb[:, 1], in_=kt8[:, 1])
        nc.scalar.dma_start(out=qt8_sb[:, 1], in_=qt8[:, 1])
    nc.gpsimd.dma_start(out=ve_sb[:, 2], in_=ve[:, 2])
    nc.gpsimd.dma_start(out=ve_sb[:, 3], in_=ve[:, 3])

    def emit_scores(g, t, k, qlo, w, s_t, fp8):
        if fp8:
            # qt8 holds only the upper query half; its column x = q - HALF
            base = 32 * t
            for c0 in range(0, w, 512):
                c1 = min(c0 + 512, w)
                nc.tensor.matmul(
                    s_t[:, c0:c1],
                    lhsT=kt8_sb[base:base + 32, g, :, P * k:P * (k + 1)],
                    rhs=qt8_sb[base:base + 32, g, :, qlo - HALF + c0:qlo - HALF + c1],
                    start=True,
                    stop=True,
                    perf_mode=mybir.MatmulPerfMode.DoubleRow,
                    skip_group_check=True,
                )
        else:
            bp = 64 * t
            for c0 in range(0, w, 512):
                c1 = min(c0 + 512, w)
                nc.tensor.matmul(
                    s_t[:, c0:c1],
                    lhsT=kt_sb[bp:bp + 64, g, P * k:P * (k + 1)],
                    rhs=qt_sb[bp:bp + 64, g, qlo + c0:qlo + c1],
                    start=True,
                    stop=True,
                    skip_group_check=True,
                )

    for g in range(PAIRS):
        for pss in range(2):
            q0 = pss * HALF
            q1 = q0 + HALF
            kmax = 8 * (pss + 1)
            j0 = 8 * pss
            pvA = [
                pv_psum.tile([P, 7 * DP1], F32, tag=f"pvA{t}", name=f"pvA{t}")
                for t in range(2)
            ]
            pvB = pv_psum.tile([P, 2 * DP1], F32, tag="pvB", name="pvB")
            o_acc = [
                e_pool.tile([P, 8, D], F32, tag=f"oacc{t}", name=f"oacc{t}")
                for t in range(2)
            ]

            def tile_out(t, j):
                jj = j - j0
                if jj < 7:
                    return pvA[t][:, jj * DP1:(jj + 1) * DP1]
                return pvB[:, t * DP1:(t + 1) * DP1]

            Copy = mybir.ActivationFunctionType.Copy

            def norm(dst, src, rec1):
                if pss == 1:
                    nc.scalar.activation(dst, src, Copy, scale=rec1)
                else:
                    nc.vector.tensor_scalar_mul(dst, src, rec1)

            def epilogue_A(t):
                rec = e_pool.tile([P, 7], F32, tag="rec")
                nc.vector.reciprocal(rec, pvA[t][:, D::DP1])
                for jj in range(7):
                    norm(
                        o_acc[t][:, jj, :],
                        pvA[t][:, jj * DP1:jj * DP1 + D],
                        rec[:, jj:jj + 1],
                    )

            def epilogue_B():
                rec = e_pool.tile([P, 2], F32, tag="rec2")
                nc.vector.reciprocal(rec, pvB[:, D::DP1])
                for t in range(2):
                    norm(
                        o_acc[t][:, 7, :],
                        pvB[:, t * DP1:t * DP1 + D],
                        rec[:, t:t + 1],
                    )
                # out[a, :, j0:j0+8, :] is contiguous per partition row
                nc.sync.dma_start(out=out[2 * g, :, j0:j0 + 8, :], in_=o_acc[0])
                nc.gpsimd.dma_start(out=out[2 * g + 1, :, j0:j0 + 8, :], in_=o_acc[1])

            def emit_pv(t, k, qlo, pT):
                a = 2 * g + t
                for j in range(max(j0, k), j0 + 8):
                    col = P * j - qlo
                    # start=True clears has_written for the WHOLE psum bank,
                    # so only the first matmul emitted into each bank sets it
                    if j - j0 < 7:
                        first_in_bank = k == 0 and j == j0
                    else:
                        first_in_bank = k == 0 and t == 0
                    nc.tensor.matmul(
                        tile_out(t, j),
                        lhsT=pT[:, col:col + P],
                        rhs=ve_sb[:, a, k, :],
                        start=first_in_bank,
                        stop=(k == j),
                        skip_group_check=True,
                    )
                    if k == j:
                        if j == j0 + 6:
                            epilogue_A(t)
                        elif j == j0 + 7 and t == 1:
                            epilogue_B()

            pend = [[], []]
            for k in range(kmax):
                qlo = max(q0, P * k)
                w = q1 - qlo
                diag = qlo == P * k
                schraudolph = pss == 1 and k >= KD
                for t in range(2):
                    s_t = s_psum.tile([P, HALF], F32, tag="s", name="s")
                    emit_scores(g, t, k, qlo, w, s_t, USE_FP8_SCORES and pss == 1)
                    # PV trails two chunks behind its exp so PE instructions
                    # enter the queue with satisfied deps (wait-queue depth 4)
                    if len(pend[t]) == 2:
                        emit_pv(t, *pend[t].pop(0))
                    pT = p_pool.tile([P, HALF], BF16, tag=f"p{t}", name=f"p{t}")
                    if schraudolph:
                        # exp via float-bit trick: DVE computes the int32 bit
                        # pattern, GPSIMD bitcast-copies to bf16 (diag mask
                        # fused as a multiply on the first 128 columns)
                        yi = i_pool.tile([P, HALF], I32, tag="yi", name="yi")
                        nc.vector.tensor_scalar(
                            out=yi[:, :w], in0=s_t[:, :w],
                            scalar1=float(SCH_A), scalar2=float(SCH_B),
                            op0=mybir.AluOpType.mult, op1=mybir.AluOpType.add,
                        )
                        nc.vector.tensor_copy(out=pT[:, :w], in_=yi[:, :w].bitcast(F32))
                        nc.gpsimd.affine_select(
                            out=pT[:, 0:P], in_=pT[:, 0:P],
                            compare_op=mybir.AluOpType.is_ge, fill=0.0,
                            base=0, channel_multiplier=-1, pattern=[[1, P]],
                        )
                    else:
                        nc.scalar.activation(pT[:, :w], s_t[:, :w], Exp, scale=SCALE)
                        if diag:
                            nc.gpsimd.affine_select(
                                out=pT[:, 0:P], in_=pT[:, 0:P],
                                compare_op=mybir.AluOpType.is_ge, fill=0.0,
                                base=0, channel_multiplier=-1, pattern=[[1, P]],
                            )
                    pend[t].append((k, qlo, pT))
            for t in range(2):
                for args in pend[t]:
                    emit_pv(t, *args)
                pend[t] = []


def _host_shard(inputs):
    """Build the 8 per-core input maps from full inputs (host-side numpy)."""
    q_t = np.asarray(inputs["queries_time"], dtype=np.float32)
    k_t = np.asarray(inputs["keys_time"], dtype=np.float32)
    v_t = np.asarray(inputs["values_time"], dtype=np.float32)
    q_c = np.asarray(inputs["queries_channel"], dtype=np.float32)
    k_c = np.asarray(inputs["keys_channel"], dtype=np.float32)
    v_c = np.asarray(inputs["values_channel"], dtype=np.float32)

    bf16 = ml_dtypes.bfloat16
    fp8 = ml_dtypes.float8_e4m3
    in_maps = []
    for c in range(NCORES):
        vem = np.empty((P, NATT, NKT, DP1), np.float32)
        qtm = np.empty((P, PAIRS, L), np.float32)
        ktm = np.empty((P, PAIRS, L), np.float32)
        qt8m = np.empty((64, PAIRS, 2, HALF), np.float32)
        kt8m = np.empty((64, PAIRS, 2, L), np.float32)
        for g in range(PAIRS):
            p = PAIRS * c + g
            b, h = divmod(p, H)
            qtm[:64, g, :] = q_t[b, :, h, :].T
            qtm[64:, g, :] = q_c[b, :, h, :].T
            ktm[:64, g, :] = k_t[b, :, h, :].T
            ktm[64:, g, :] = k_c[b, :, h, :].T
            for t, (qf, kf) in enumerate(((q_t, k_t), (q_c, k_c))):
                # E-index e -> partition 32*t + e%32, sub-row e//32
                qT = qf[b, HALF:, h, :].T.reshape(2, 32, HALF)
                kT = kf[b, :, h, :].T.reshape(2, 32, L)
                qt8m[32 * t:32 * (t + 1), g] = qT.transpose(1, 0, 2)
                kt8m[32 * t:32 * (t + 1), g] = kT.transpose(1, 0, 2)
            for t, v_full in enumerate((v_t, v_c)):
                a = 2 * g + t
                vem[:, a, :, :D] = (
                    v_full[b, :, h, :].reshape(NKT, P, D).transpose(1, 0, 2)
                )
                vem[:, a, :, D] = 1.0
        m = {
            "qt": np.ascontiguousarray(qtm).astype(bf16),
            "kt": np.ascontiguousarray(ktm).astype(bf16),
            "ve": np.ascontiguousarray(vem).astype(bf16),
        }
        if USE_FP8_SCORES:
            m["qt8"] = np.ascontiguousarray(qt8m).astype(fp8)
            m["kt8"] = np.ascontiguousarray(kt8m).astype(fp8)
        in_maps.append(m)
    return in_maps


def _run(in_maps, trace=False):
    if "nc" not in _CACHE:
        _CACHE["nc"] = _build_nc()
    return run_bass_kernel_spmd(
        _CACHE["nc"], in_maps, core_ids=list(range(NCORES)), trace=trace
    )


def kernel(**inputs):
    in_maps = _host_shard(inputs)
    res = _run(in_maps, trace=False)
    v_time = np.empty((B, L, H, D), np.float32)
    v_chan = np.empty((B, L, H, D), np.float32)
    for c in range(NCORES):
        o = np.asarray(res.results[c]["out"])  # [NATT, P, NKT, D]
        for g in range(PAIRS):
            p = PAIRS * c + g
            b, h = divmod(p, H)
            # q = 128*j + qq lives at o[a, qq, j, :]
            v_time[b, :, h, :] = o[2 * g + 0].transpose(1, 0, 2).reshape(L, D)
            v_chan[b, :, h, :] = o[2 * g + 1].transpose(1, 0, 2).reshape(L, D)
    return v_time, v_chan


# revision 15
# speedup vs baseline: 1.0197x; 1.0197x over previous
"""AnomalyAttention (two causal attentions per (b,h)) on 8 TRN2 NeuronCores.

Sharding: B*H = 16 (batch, head) pairs -> 2 pairs per core. Each core runs
4 independent causal attentions (time + channel for each of its 2 pairs).
No cross-core communication.

Layout ("transposed PV" + dual-engine exp), bf16 matmuls:
  S^T[k, q]   = kT_tile.T @ qT        (PE bf16, contraction E=64; the two
                attn types use PE row groups 0-63 / 64-127)
  P^T         = exp(scale * S^T)      two paths:
                - ACT exp for pass-0 and pass-1 early key tiles
                - DVE Schraudolph for pass-1 k>=8: tensor_scalar
                  (x*a+b -> int32 = float bits of exp(x)) + bitcast
                  tensor_copy -> bf16 (~3.5% rel err, but only on the
                  late-key mass of late queries whose outputs are small)
  diag mask   = GPSIMD affine_select zero-triangle on the diag block
  out[q, d]  += P^T[k, q-blk].T @ V_ext[k, d]   (PE bf16, 65-col moving;
                PV emitted two chunks behind exp so PE instructions queue
                with satisfied deps)
V_ext carries a ones column -> out col 64 = softmax denominator per query
partition. Epilogue per completed PSUM bank: DVE reciprocal + per-q-tile
tensor_scalar multiply into a staging tile; one contiguous DMA per 8
q-tiles. PSUM discipline: start=True clears has_written for the WHOLE
bank, so exactly one start per bank (first matmul emitted into it).
"""

import math
from contextlib import ExitStack

import ml_dtypes
import numpy as np

import concourse.bacc as bacc
import concourse.mybir as mybir
import concourse.tile as tile
from concourse.bass_utils import run_bass_kernel_spmd

B, L, H, E, D = 2, 2048, 8, 64, 64
NCORES = 8
PAIRS = (B * H) // NCORES          # (b,h) pairs per core = 2
NATT = 2 * PAIRS                   # attentions per core = 4
SCALE = 1.0 / math.sqrt(E)
P = 128                            # partitions / key-tile size
NKT = L // P                       # 16 key tiles
HALF = L // 2                      # query half-pass size (PSUM budget)
DP1 = D + 1                        # value cols + denominator ones-column
F32 = mybir.dt.float32
I32 = mybir.dt.int32
BF16 = mybir.dt.bfloat16
FP8 = mybir.dt.float8e4

# fp8 DoubleRow scores only for pass 1 (query rows >= 1024): early rows
# have few softmax terms and large outputs, so they keep bf16 exactness;
# late rows average over >=1024 keys and absorb the fp8 quantization.
USE_FP8_SCORES = False
# Schraudolph exp on DVE for pass-1 key tiles >= KD (late keys of late
# queries; their extra ~3.5% relative error lands on the rows with the
# smallest output magnitude)
KD = 8

LOG2E = 1.4426950408889634
SCH_A = SCALE * LOG2E * (1 << 23)
SCH_B = float((127 << 23) - 298634)

_CACHE = {}


def _build_nc():
    nc = bacc.Bacc()
    qt = nc.declare_dram_parameter("qt", [P, PAIRS, L], BF16, isOutput=False)
    kt = nc.declare_dram_parameter("kt", [P, PAIRS, L], BF16, isOutput=False)
    if USE_FP8_SCORES:
        # E-dim folded as [32 partitions, 2 sub-rows] for DoubleRow
        qt8 = nc.declare_dram_parameter("qt8", [64, PAIRS, 2, HALF], FP8, isOutput=False)
        kt8 = nc.declare_dram_parameter("kt8", [64, PAIRS, 2, L], FP8, isOutput=False)
    else:
        qt8 = kt8 = None
    ve = nc.declare_dram_parameter("ve", [P, NATT, NKT, DP1], BF16, isOutput=False)
    out = nc.declare_dram_parameter("out", [NATT, P, NKT, D], F32, isOutput=True)

    with tile.TileContext(nc) as tc:
        with ExitStack() as ctx:
            _body(ctx, tc, qt, kt, qt8, kt8, ve, out)
    nc.finalize()
    return nc


def _body(ctx, tc, qt, kt, qt8, kt8, ve, out):
    nc = tc.nc
    Exp = mybir.ActivationFunctionType.Exp

    persist = ctx.enter_context(tc.tile_pool(name="persist", bufs=1))
    s_psum = ctx.enter_context(tc.tile_pool(name="s_psum", bufs=2, space="PSUM"))
    pv_psum = ctx.enter_context(tc.tile_pool(name="pv_psum", bufs=1, space="PSUM"))
    p_pool = ctx.enter_context(tc.tile_pool(name="p_pool", bufs=4))
    i_pool = ctx.enter_context(tc.tile_pool(name="i_pool", bufs=3))
    e_pool = ctx.enter_context(tc.tile_pool(name="e_pool", bufs=4))

    # warm the ACT exp table before any real dependency exists
    warm = persist.tile([1, 8], F32)
    nc.vector.memset(warm, 0.0)
    nc.scalar.activation(warm, warm, Exp)


    # triangular mask tile: 1 where q >= key else 0 (built once)
    mask = persist.tile([P, P], BF16)
    nc.vector.memset(mask, 1.0)
    nc.gpsimd.affine_select(
        out=mask, in_=mask, compare_op=mybir.AluOpType.is_ge,
        fill=0.0, base=0, channel_multiplier=-1, pattern=[[1, P]],
    )

    qt_sb = persist.tile([P, PAIRS, L], BF16)
    kt_sb = persist.tile([P, PAIRS, L], BF16)
    if USE_FP8_SCORES:
        qt8_sb = persist.tile([64, PAIRS, 2, HALF], FP8)
        kt8_sb = persist.tile([64, PAIRS, 2, L], FP8)
    ve_sb = persist.tile([P, NATT, NKT, DP1], BF16)

    # staged input DMA: the three pieces the first chunk needs go out on
    # three different engines' queues in parallel
    nc.gpsimd.dma_start(out=kt_sb[:, 0, 0:P], in_=kt[:, 0, 0:P])
    nc.scalar.dma_start(out=qt_sb[:, 0, 0:512], in_=qt[:, 0, 0:512])
    nc.sync.dma_start(out=qt_sb[:, 0, 512:HALF], in_=qt[:, 0, 512:HALF])
    nc.gpsimd.dma_start(out=ve_sb[:, 0], in_=ve[:, 0])
    nc.gpsimd.dma_start(out=ve_sb[:, 1], in_=ve[:, 1])
    nc.sync.dma_start(out=kt_sb[:, 0, P:L], in_=kt[:, 0, P:L])
    nc.scalar.dma_start(out=qt_sb[:, 0, HALF:L], in_=qt[:, 0, HALF:L])
    if USE_FP8_SCORES:
        nc.sync.dma_start(out=kt8_sb[:, 0], in_=kt8[:, 0])
        nc.scalar.dma_start(out=qt8_sb[:, 0], in_=qt8[:, 0])
    nc.sync.dma_start(out=kt_sb[:, 1], in_=kt[:, 1])
    nc.scalar.dma_start(out=qt_sb[:, 1], in_=qt[:, 1])
    if USE_FP8_SCORES:
        nc.sync.dma_start(out=kt8_sb[:, 1], in_=kt8[:, 1])
        nc.scalar.dma_start(out=qt8_sb[:, 1], in_=qt8[:, 1])
    nc.gpsimd.dma_start(out=ve_sb[:, 2], in_=ve[:, 2])
    nc.gpsimd.dma_start(out=ve_sb[:, 3], in_=ve[:, 3])

    def emit_scores(g, t, k, qlo, w, s_t, fp8):
        if fp8:
            # qt8 holds only the upper query half; its column x = q - HALF
            base = 32 * t
            for c0 in range(0, w, 512):
                c1 = min(c0 + 512, w)
                nc.tensor.matmul(
                    s_t[:, c0:c1],
                    lhsT=kt8_sb[base:base + 32, g, :, P * k:P * (k + 1)],
                    rhs=qt8_sb[base:base + 32, g, :, qlo - HALF + c0:qlo - HALF + c1],
                    start=True,
                    stop=True,
                    perf_mode=mybir.MatmulPerfMode.DoubleRow,
                    skip_group_check=True,
                )
        else:
            bp = 64 * t
            for c0 in range(0, w, 512):
                c1 = min(c0 + 512, w)
                nc.tensor.matmul(
                    s_t[:, c0:c1],
                    lhsT=kt_sb[bp:bp + 64, g, P * k:P * (k + 1)],
                    rhs=qt_sb[bp:bp + 64, g, qlo + c0:qlo + c1],
                    start=True,
                    stop=True,
                    skip_group_check=True,
                )

    for g in range(PAIRS):
        for pss in range(2):
            q0 = pss * HALF
            q1 = q0 + HALF
            kmax = 8 * (pss + 1)
            j0 = 8 * pss
            pvA = [
                pv_psum.tile([P, 7 * DP1], F32, tag=f"pvA{t}", name=f"pvA{t}")
                for t in range(2)
            ]
            pvB = pv_psum.tile([P, 2 * DP1], F32, tag="pvB", name="pvB")
            o_acc = [
                e_pool.tile([P, 8, D], F32, tag=f"oacc{t}", name=f"oacc{t}")
                for t in range(2)
            ]

            def tile_out(t, j):
                jj = j - j0
                if jj < 7:
                    return pvA[t][:, jj * DP1:(jj + 1) * DP1]
                return pvB[:, t * DP1:(t + 1) * DP1]

            def norm(dst, src, rec1):
                nc.vector.tensor_scalar_mul(dst, src, rec1)

            def epilogue_A(t):
                rec = e_pool.tile([P, 7], F32, tag="rec")
                nc.vector.reciprocal(rec, pvA[t][:, D::DP1])
                for jj in range(7):
                    norm(
                        o_acc[t][:, jj, :],
                        pvA[t][:, jj * DP1:jj * DP1 + D],
                        rec[:, jj:jj + 1],
                    )

            def epilogue_B():
                rec = e_pool.tile([P, 2], F32, tag="rec2")
                nc.vector.reciprocal(rec, pvB[:, D::DP1])
                for t in range(2):
                    norm(
                        o_acc[t][:, 7, :],
                        pvB[:, t * DP1:t * DP1 + D],
                        rec[:, t:t + 1],
                    )
                # out[a, :, j0:j0+8, :] is contiguous per partition row
                nc.sync.dma_start(out=out[2 * g, :, j0:j0 + 8, :], in_=o_acc[0])
                nc.gpsimd.dma_start(out=out[2 * g + 1, :, j0:j0 + 8, :], in_=o_acc[1])

            def emit_pv(t, k, qlo, pT):
                a = 2 * g + t
                for j in range(max(j0, k), j0 + 8):
                    col = P * j - qlo
                    # start=True clears has_written for the WHOLE psum bank,
                    # so only the first matmul emitted into each bank sets it
                    if j - j0 < 7:
                        first_in_bank = k == 0 and j == j0
                    else:
                        first_in_bank = k == 0 and t == 0
                    nc.tensor.matmul(
                        tile_out(t, j),
                        lhsT=pT[:, col:col + P],
                        rhs=ve_sb[:, a, k, :],
                        start=first_in_bank,
                        stop=(k == j),
                        skip_group_check=True,
                    )
                    if k == j:
                        if j == j0 + 6:
                            epilogue_A(t)
                        elif j == j0 + 7 and t == 1:
                            epilogue_B()

            pend = [[], []]
            for k in range(kmax):
                qlo = max(q0, P * k)
                w = q1 - qlo
                diag = qlo == P * k
                schraudolph = pss == 1 and k >= KD
                for t in range(2):
                    s_t = s_psum.tile([P, HALF], F32, tag="s", name="s")
                    emit_scores(g, t, k, qlo, w, s_t, USE_FP8_SCORES and pss == 1)
                    # PV trails two chunks behind its exp so PE instructions
                    # enter the queue with satisfied deps (wait-queue depth 4)
                    if len(pend[t]) == 2:
                        emit_pv(t, *pend[t].pop(0))
                    pT = p_pool.tile([P, HALF], BF16, tag=f"p{t}", name=f"p{t}")
                    if schraudolph:
                        # exp via float-bit trick: DVE computes the int32 bit
                        # pattern, GPSIMD bitcast-copies to bf16 (diag mask
                        # fused as a multiply on the first 128 columns)
                        yi = i_pool.tile([P, HALF], I32, tag="yi", name="yi")
                        nc.vector.tensor_scalar(
                            out=yi[:, :w], in0=s_t[:, :w],
                            scalar1=float(SCH_A), scalar2=float(SCH_B),
                            op0=mybir.AluOpType.mult, op1=mybir.AluOpType.add,
                        )
                        nc.vector.tensor_copy(out=pT[:, :w], in_=yi[:, :w].bitcast(F32))
                        nc.gpsimd.affine_select(
                            out=pT[:, 0:P], in_=pT[:, 0:P],
                            compare_op=mybir.AluOpType.is_ge, fill=0.0,
                            base=0, channel_multiplier=-1, pattern=[[1, P]],
                        )
                    else:
                        nc.scalar.activation(pT[:, :w], s_t[:, :w], Exp, scale=SCALE)
                        if diag:
                            nc.gpsimd.affine_select(
                                out=pT[:, 0:P], in_=pT[:, 0:P],
                                compare_op=mybir.AluOpType.is_ge, fill=0.0,
                                base=0, channel_multiplier=-1, pattern=[[1, P]],
                            )
                    pend[t].append((k, qlo, pT))
            for t in range(2):
                for args in pend[t]:
                    emit_pv(t, *args)
                pend[t] = []


def _host_shard(inputs):
    """Build the 8 per-core input maps from full inputs (host-side numpy)."""
    q_t = np.asarray(inputs["queries_time"], dtype=np.float32)
    k_t = np.asarray(inputs["keys_time"], dtype=np.float32)
    v_t = np.asarray(inputs["values_time"], dtype=np.float32)
    q_c = np.asarray(inputs["queries_channel"], dtype=np.float32)
    k_c = np.asarray(inputs["keys_channel"], dtype=np.float32)
    v_c = np.asarray(inputs["values_channel"], dtype=np.float32)

    bf16 = ml_dtypes.bfloat16
    fp8 = ml_dtypes.float8_e4m3
    in_maps = []
    for c in range(NCORES):
        vem = np.empty((P, NATT, NKT, DP1), np.float32)
        qtm = np.empty((P, PAIRS, L), np.float32)
        ktm = np.empty((P, PAIRS, L), np.float32)
        qt8m = np.empty((64, PAIRS, 2, HALF), np.float32)
        kt8m = np.empty((64, PAIRS, 2, L), np.float32)
        for g in range(PAIRS):
            p = PAIRS * c + g
            b, h = divmod(p, H)
            qtm[:64, g, :] = q_t[b, :, h, :].T
            qtm[64:, g, :] = q_c[b, :, h, :].T
            ktm[:64, g, :] = k_t[b, :, h, :].T
            ktm[64:, g, :] = k_c[b, :, h, :].T
            for t, (qf, kf) in enumerate(((q_t, k_t), (q_c, k_c))):
                # E-index e -> partition 32*t + e%32, sub-row e//32
                qT = qf[b, HALF:, h, :].T.reshape(2, 32, HALF)
                kT = kf[b, :, h, :].T.reshape(2, 32, L)
                qt8m[32 * t:32 * (t + 1), g] = qT.transpose(1, 0, 2)
                kt8m[32 * t:32 * (t + 1), g] = kT.transpose(1, 0, 2)
            for t, v_full in enumerate((v_t, v_c)):
                a = 2 * g + t
                vem[:, a, :, :D] = (
                    v_full[b, :, h, :].reshape(NKT, P, D).transpose(1, 0, 2)
                )
                vem[:, a, :, D] = 1.0
        m = {
            "qt": np.ascontiguousarray(qtm).astype(bf16),
            "kt": np.ascontiguousarray(ktm).astype(bf16),
            "ve": np.ascontiguousarray(vem).astype(bf16),
        }
        if USE_FP8_SCORES:
            m["qt8"] = np.ascontiguousarray(qt8m).astype(fp8)
            m["kt8"] = np.ascontiguousarray(kt8m).astype(fp8)
        in_maps.append(m)
    return in_maps


def _run(in_maps, trace=False):
    if "nc" not in _CACHE:
        _CACHE["nc"] = _build_nc()
    return run_bass_kernel_spmd(
        _CACHE["nc"], in_maps, core_ids=list(range(NCORES)), trace=trace
    )


def kernel(**inputs):
    in_maps = _host_shard(inputs)
    res = _run(in_maps, trace=False)
    v_time = np.empty((B, L, H, D), np.float32)
    v_chan = np.empty((B, L, H, D), np.float32)
    for c in range(NCORES):
        o = np.asarray(res.results[c]["out"])  # [NATT, P, NKT, D]
        for g in range(PAIRS):
            p = PAIRS * c + g
            b, h = divmod(p, H)
            # q = 128*j + qq lives at o[a, qq, j, :]
            v_time[b, :, h, :] = o[2 * g + 0].transpose(1, 0, 2).reshape(L, D)
            v_chan[b, :, h, :] = o[2 * g + 1].transpose(1, 0, 2).reshape(L, D)
    return v_time, v_chan
